# revision 63
# baseline (speedup 1.0000x reference)
"""AdaptiveGeometryAttention Trainium2 kernel (8 NeuronCores).

Sharding: core c handles batch b = c//4 and head group hg = c%4 (4 of 16 heads).
Each core computes its heads' attention and a partial out-projection (T, E);
the host sums the 4 partials per batch.

Key algebraic restructurings vs the reference:
  - The Lorentz inner product -<q_hyp, k_hyp>_L is a single K=65 matmul over
    augmented vectors [-gf*q_d | tim] x [gf*k_d | tim] with the k-side space
    row 0 zeroed in the broadcast selector (cancels the discarded r0 row).
  - arccosh(m)^2 ~= A*tanh(sf*(m-1)+bf) + a*(m-1) + const. Constants and
    per-query additive terms cancel in softmax and are dropped; the blend is
    computed re-centered as G = psU - gamma*(F-1) so exp(G) stays in fp16
    range. The -a*m linear term and the (1-alpha)/8 euclid scale fold into
    a second accumulated matmul (per-query column scaling of q-side operands).
  - The per-query scale rows (beta/-ag/gA) are affine in tanh(alpha-logit/2);
    the affines fold into the PE broadcast selectors against [tA; ones].
  - Softmax denominators come from a ones-column appended to V; spike masking
    and 1/Z fold into one per-query scale applied to y^T before out-proj.
  - psM and the hyper-U matmul share the KH stationary operand (consecutive).
  - Emission is software-pipelined; the out-projection of query block 0 runs
    under the j=1 score groups and its DMA streams before the tail.
"""

import sys
import contextlib

sys.path.insert(0, "/opt/trn_rl_repo")

import numpy as np

B, T, E, H = 2, 1024, 1024, 16
D = 64
NCORES = 8
HPC = 4  # heads per core

# arccosh(1+x)^2 fit on x in [0, 2.2]
A_FIT = 54.32641203
S_FIT = 0.28607594936708863
B_FIT = 2.0
A_LIN = 0.8910533
# sqrt(1+w) deg-3 fit on w in [0, 0.95]
SQ3, SQ2, SQ1, SQ0 = 0.02492195, -0.10732602, 0.49672154, 1.00014421

# host-derived scalars, set by kernel() before _build()
_S2 = _S2M2 = _SS = _THR = _BADJ = 0.0
_DEBUG = False


def _build():
    from concourse import bass, mybir, tile, bacc

    F32 = mybir.dt.float32
    F32R = mybir.dt.float32r
    F16 = mybir.dt.float16
    BF16 = mybir.dt.bfloat16
    AF = mybir.ActivationFunctionType
    OP = mybir.AluOpType

    def r(ap):
        return ap.bitcast(F32R)

    nc = bacc.Bacc()

    xT = nc.declare_dram_parameter("xT", [E, T], F32R, isOutput=False)
    wqk = nc.declare_dram_parameter("wqk", [E, 512], F32R, isOutput=False)
    wv = nc.declare_dram_parameter("wv", [E, 256], F32R, isOutput=False)
    wai = nc.declare_dram_parameter("wai", [E, 5], F32R, isOutput=False)
    bqk = nc.declare_dram_parameter("bqk", [128, 4], F32, isOutput=False)
    bqk2 = nc.declare_dram_parameter("bqk2", [64, 8], F32, isOutput=False)
    bvT = nc.declare_dram_parameter("bvT", [1, 256], F32R, isOutput=False)
    wo = nc.declare_dram_parameter("wo", [256, E], F16, isOutput=False)
    onesel = nc.declare_dram_parameter("onesel", [128, 4, 40], F32R, isOutput=False)
    cst = nc.declare_dram_parameter("cst", [5, 1], F32, isOutput=False)
    sel8 = nc.declare_dram_parameter("sel8", [8, 512], F32R, isOutput=False)
    ones1 = nc.declare_dram_parameter("ones1", [1, T], F32, isOutput=False)
    selu = nc.declare_dram_parameter("selu", [6, 512], F32R, isOutput=False)
    selg = nc.declare_dram_parameter("selg", [6, 512], F32R, isOutput=False)
    tri = nc.declare_dram_parameter("tri", [128, 128], BF16, isOutput=False)
    out = nc.declare_dram_parameter("out", [128, 8, E], F16, isOutput=True)
    if _DEBUG:
        d_y0 = nc.declare_dram_parameter("d_y0", [128, T], F16, isOutput=True)
        d_y1 = nc.declare_dram_parameter("d_y1", [128, T], F16, isOutput=True)

    with tile.TileContext(nc) as tc:
        ctx = contextlib.ExitStack()
        with ctx:
            main = ctx.enter_context(tc.tile_pool(name="main", bufs=1))

            # ---- persistent small inputs ----
            tSEL = main.tile([128, 4, 40], F32R)
            tBQK = main.tile([128, 4], F32)
            tBQK2 = main.tile([64, 8], F32)
            tCST = main.tile([5, 1], F32)
            tBVT = main.tile([1, 256], F32R)
            tS8 = main.tile([8, 512], F32R)
            tSU = main.tile([6, 512], F32R)
            tSG = main.tile([6, 512], F32R)
            tTRI = main.tile([128, 128], BF16)

            tONEf = main.tile([1, 128], F32)
            tONE = main.tile([1, 128], F32R)
            tBADJ = main.tile([128, 1], F32)

            # ---- persistent state ----
            tQ = [main.tile([64, T], F16, name=f"tQ{h}") for h in range(HPC)]
            # BIGS[:, h, :]: row 0 tim_k, rows 1:64 gf_k*k[1:], 64:128 k+bias
            BIGS = main.tile([128, HPC, T], F16)
            # QHL[:, h]: row 0 tim_q, rows 1:64 -gf_q*q[1:]
            QHL = main.tile([64, HPC, T], F16)
            # MU[:, h]: rows 0:64 -ag*QHL, rows 64:128 beta*q
            MU = main.tile([128, HPC, T], F16)
            GAB = {h: main.tile([128, T], F16, name=f"GAB{h}")
                   for h in range(HPC)}
            tV = main.tile([128, 8, HPC, 65], BF16)
            Pgf = main.tile([8, T], F32R)  # rows 0-3 gf_q(h), 4-7 gf_k(h)
            Ptim = main.tile([8, T], F16)  # rows 0-3 tim_q(h), 4-7 tim_k(h)
            tA6 = main.tile([6, T], F32)   # rows 0-3 tanh, 4 imp, 5 ones
            tA6f = main.tile([6, T], F32R)  # fp32r-rounded copy for the PE
            spk = main.tile([1, T], F32)
            tY0 = main.tile([128, T], F16)
            tY1 = main.tile([128, T], F16)
            tYL = [tY0, tY1]
            tWO = main.tile([128, 2, E], F16)
            OO = main.tile([128, 4, E], F16)  # out-proj staging (reused)

            def prep_body(h, cc, pool):
                csl = slice(cc * 512, (cc + 1) * 512)
                pb = pool.tile([128, 512], F32, tag="psb")
                nc.tensor.matmul(pb[:], tS8[:, h * 128:(h + 1) * 128],
                                 Pgf[:, csl], start=True, stop=True)
                # rows 0 get garbage (q0/k0 products); the time-row DMAs
                # below overwrite them
                nc.vector.tensor_mul(QHL[0:64, h, csl], pb[0:64, :],
                                     tQ[h][:, csl])
                nc.vector.tensor_mul(BIGS[0:64, h, csl], pb[64:128, :],
                                     BIGS[64:128, h, csl])
                nc.sync.dma_start(out=QHL[0:1, h, csl],
                                  in_=Ptim[h:h + 1, csl])
                nc.sync.dma_start(out=BIGS[0:1, h, csl],
                                  in_=Ptim[4 + h:5 + h, csl])
                pbu = pool.tile([128, 512], F32, tag="psb")
                nc.tensor.matmul(pbu[:], tSU[:, h * 128:(h + 1) * 128],
                                 tA6f[:, csl], start=True, stop=True)
                nc.vector.tensor_mul(MU[0:64, h, csl], pbu[0:64, :],
                                     QHL[0:64, h, csl])
                nc.vector.tensor_mul(MU[64:128, h, csl], pbu[64:128, :],
                                     tQ[h][:, csl])
                pb4 = pool.tile([128, 512], F32, tag="psb")
                nc.tensor.matmul(pb4[:], tSG[:, h * 128:(h + 1) * 128],
                                 tA6f[:, csl], start=True, stop=True)
                nc.scalar.copy(out=GAB[h][:, csl], in_=pb4[:])

            # ================= projection phase =================
            with tc.tile_pool(name="pin", bufs=1) as pin, \
                 tc.tile_pool(name="ppj2", bufs=1, space="PSUM") as ppj2:
                tXT = pin.tile([128, 8, T], F32R)
                tWQK = pin.tile([128, 8, 512], F32R)
                tWV = pin.tile([128, 8, 256], F32R)
                tWAI = pin.tile([128, 8, 5], F32R)
                RW = pin.tile([8, 4, T], F32)
                SPK5 = pin.tile([5, T], F32)
                for k in range(8):
                    nc.gpsimd.dma_start(out=tXT[:, k, :],
                                        in_=xT[k * 128:(k + 1) * 128, :])
                    nc.sync.dma_start(out=tWQK[:, k, :],
                                      in_=wqk[k * 128:(k + 1) * 128, :])
                for k in range(8):
                    nc.scalar.dma_start(out=tWAI[:, k, :],
                                        in_=wai[k * 128:(k + 1) * 128, :])


                # small constants on the sync queue, after the wqk loads
                nc.sync.dma_start(out=tBQK[:], in_=bqk[:])
                nc.sync.dma_start(out=tBQK2[:], in_=bqk2[:])
                nc.sync.dma_start(out=tCST[:], in_=cst[:])
                nc.sync.dma_start(out=tSEL[:], in_=onesel[:])
                nc.sync.dma_start(out=tBVT[:], in_=bvT[:])
                nc.sync.dma_start(out=tS8[:], in_=sel8[:])
                nc.sync.dma_start(out=tSU[:], in_=selu[:])
                nc.sync.dma_start(out=tSG[:], in_=selg[:])
                nc.sync.dma_start(out=tTRI[:], in_=tri[:])
                for k in range(8):
                    nc.sync.dma_start(out=tWV[:, k, :],
                                      in_=wv[k * 128:(k + 1) * 128, :])
                for g in range(2):
                    nc.sync.dma_start(out=tWO[:, g, :],
                                      in_=wo[g * 128:(g + 1) * 128, :])
                nc.vector.memset(tONEf[:], 1.0)
                nc.vector.tensor_copy(out=tONE[:], in_=tONEf[:])
                nc.vector.memset(tBADJ[:], _BADJ)
                nc.sync.dma_start(out=tA6[5:6, :], in_=ones1[:])

                psNZ = ppj2.tile([40, T], F32, tag="nz")
                with tc.tile_pool(name="ppj", bufs=2, space="PSUM") as ppj:
                    for h in range(HPC):
                        ps = ppj.tile([128, T], F32, tag="psqk")
                        for k in range(8):
                            for n in range(2):
                                nc.tensor.matmul(
                                    ps[:, n * 512:(n + 1) * 512],
                                    r(tWQK[:, k, h * 128:(h + 1) * 128]),
                                    r(tXT[:, k, n * 512:(n + 1) * 512]),
                                    start=(k == 0), stop=(k == 7),
                                )
                        sq = pin.tile([128, T], F32R, tag="sq", bufs=2)
                        nc.scalar.activation(out=sq[:], in_=ps[:], func=AF.Square,
                                             bias=tBQK[:, h:h + 1])
                        nc.scalar.activation(out=tQ[h][:], in_=ps[0:64, :],
                                             func=AF.Identity,
                                             bias=tBQK2[:, h:h + 1])
                        nc.vector.tensor_scalar(BIGS[64:128, h, :],
                                                ps[64:128, :],
                                                tBQK2[:, 4 + h:5 + h], None,
                                                op0=OP.add)
                        for n in range(2):
                            nc.tensor.matmul(
                                psNZ[:, n * 512:(n + 1) * 512],
                                r(tSEL[:, h, 0:40]),
                                r(sq[:, n * 512:(n + 1) * 512]),
                                start=(h == 0), stop=(h == HPC - 1),
                            )

                psA = ppj2.tile([5, T], F32, tag="alpha")
                for k in range(8):
                    for n in range(2):
                        nc.tensor.matmul(
                            psA[:, n * 512:(n + 1) * 512],
                            r(tWAI[:, k, :]),
                            r(tXT[:, k, n * 512:(n + 1) * 512]),
                            start=(k == 0), stop=(k == 7),
                        )

                nc.scalar.activation(out=tA6[0:5, :], in_=psA[:], func=AF.Tanh,
                                     scale=0.5, bias=tCST[0:5, :])
                nc.vector.tensor_scalar(SPK5[:], psA[0:5, :], _THR, None,
                                        op0=OP.is_gt)
                nc.scalar.dma_start(out=spk[:], in_=SPK5[4:5, :])
                nc.vector.tensor_copy(out=tA6f[:], in_=tA6[:])

                # ---- row quantities: PSUM-touching head on vector, the
                # long serial tails run on vector (cc0) / gpsimd (cc1) in
                # parallel ----
                def row_head(cc):
                    cl = slice(cc * 512, (cc + 1) * 512)
                    sA = RW[0:8, 0, cl]
                    sB = RW[0:8, 1, cl]
                    sC = RW[0:8, 2, cl]
                    n2 = psNZ[0:8, cl]
                    z2 = psNZ[32:40, cl]
                    nc.vector.tensor_scalar_max(sA, n2, 1e-24)
                    nc.vector.reciprocal_approx_fast(out=sB, in_=sA)      # 1/n2
                    nc.vector.tensor_mul(sC, z2, sB)                      # q2n

                def row_tail(cc, eng):
                    cl = slice(cc * 512, (cc + 1) * 512)
                    sA = RW[0:8, 0, cl]
                    sB = RW[0:8, 1, cl]
                    sC = RW[0:8, 2, cl]
                    sD = RW[0:8, 3, cl]
                    gfc = Pgf[0:8, cl]
                    timc = Ptim[0:8, cl]
                    nc.scalar.activation(out=gfc, in_=sB, func=AF.Sqrt)   # invn
                    eng.tensor_scalar(sA, sC, _S2M2, _S2, op0=OP.mult,
                                      op1=OP.add)
                    eng.tensor_scalar_max(sA, sA, 1e-8)                   # y = nu^2
                    # f = sinh(nu)/nu ~= (y/120 + 1/6)*y + 1  (y <= s^2 < 0.7)
                    eng.tensor_scalar(sD, sA, 1.0 / 120.0, 1.0 / 6.0,
                                      op0=OP.mult, op1=OP.add)
                    eng.tensor_mul(sD, sD, sA)
                    eng.tensor_scalar_add(sA, sD, 1.0)                    # f
                    eng.scalar_tensor_tensor(out=gfc, in0=gfc, scalar=_SS,
                                             in1=sA, op0=OP.mult, op1=OP.mult)
                    eng.tensor_scalar(sB, sC, -_S2, _S2, op0=OP.mult,
                                      op1=OP.add)
                    eng.tensor_mul(sC, sA, sA)                            # f^2
                    eng.tensor_mul(sB, sC, sB)                            # w
                    # tim = sqrt(1+w) ~= (d2*w + d1)*w + d0 on w in [0, 0.8]
                    eng.tensor_scalar(sC, sB, -0.07717365, 0.48697457,
                                      op0=OP.mult, op1=OP.add)
                    eng.tensor_mul(sC, sC, sB)
                    eng.tensor_scalar_add(timc, sC, 1.00080169)           # time

                row_head(0)
                row_tail(0, nc.vector)
                row_head(1)

                # V projection (k-outer, PSUM-resident accumulators)
                tVonef = pin.tile([128, 32], F32)
                nc.vector.memset(tVonef[:], 1.0)
                nc.vector.tensor_copy(out=tV[:, :, :, 64:65], in_=tVonef[:])
                with tc.tile_pool(name="ppv", bufs=1, space="PSUM") as ppv:
                    psvAll = ppv.tile([128, 8, 256], F32, tag="psv")
                    for m in range(8):
                        nc.tensor.matmul(psvAll[:, m, :], r(tONE[:]), r(tBVT[:]),
                                         start=True, stop=False)
                    for k in range(8):
                        for m in range(8):
                            nc.tensor.matmul(
                                psvAll[:, m, :],
                                r(tXT[:, k, m * 128:(m + 1) * 128]),
                                r(tWV[:, k, :]),
                                start=False, stop=(k == 7),
                            )
                    for m in range(8):
                        src = psvAll[:, m, :].rearrange("p (h d) -> p h d", h=HPC)
                        nc.scalar.copy(out=tV[:, m, :, 0:64], in_=src)

                with tc.tile_pool(name="ppx", bufs=2, space="PSUM") as ppx:
                    prep_body(0, 0, ppx)
                    prep_body(1, 0, ppx)
                    row_tail(1, nc.vector)



            # ================= attention =================
            jsl = [slice(0, 512), slice(512, 1024)]
            with tc.tile_pool(name="ap", bufs=1) as apool, \
                 tc.tile_pool(name="hp", bufs=2) as hp, \
                 tc.tile_pool(name="pps", bufs=2, space="PSUM") as pps, \
                 tc.tile_pool(name="ppy", bufs=2, space="PSUM") as ppy, \
                 tc.tile_pool(name="ppb", bufs=2, space="PSUM") as ppb:

                def prep(h, cc):
                    prep_body(h, cc, ppb)

                def scores(h, j):
                    nsb = 4 * (j + 1)
                    PTJ = hp.tile([128, 8, 512], BF16, tag="PTJ", bufs=3)
                    for sb in range(nsb):
                        o = max(0, 128 * sb - 512 * j)
                        W = 512 - o
                        c0 = 512 * j + o
                        psM = pps.tile([128, 512], F32, tag="psM")
                        psU = pps.tile([128, 512], F32, tag="psU")
                        nc.tensor.matmul(
                            psM[:, o:512],
                            BIGS[0:64, h, sb * 128:(sb + 1) * 128],
                            QHL[0:64, h, c0:c0 + W],
                            start=True, stop=True)
                        nc.tensor.matmul(
                            psU[:, o:512],
                            BIGS[0:128, h, sb * 128:(sb + 1) * 128],
                            MU[0:128, h, c0:c0 + W],
                            start=True, stop=True)
                        F = hp.tile([128, 512], F16, tag="F", bufs=4)
                        nc.scalar.activation(out=F[:, o:512], in_=psM[:, o:512],
                                             func=AF.Tanh, scale=S_FIT,
                                             bias=tBADJ[:])
                        G = hp.tile([128, 512], F16, tag="G", bufs=4)
                        # G = (F - 1) * gab  (re-centered: shifts scores by
                        # +gamma(q), a per-query constant that softmax cancels)
                        nc.gpsimd.tensor_mul(G[:, o:512], F[:, o:512],
                                             GAB[h][:, c0:c0 + W])
                        nc.vector.scalar_tensor_tensor(
                            out=G[:, o:512], in0=G[:, o:512], scalar=-1.0,
                            in1=psU[:, o:512], op0=OP.mult, op1=OP.add)
                        nc.scalar.activation(out=PTJ[:, sb, o:512],
                                             in_=G[:, o:512], func=AF.Exp)
                        if sb >= 4 * j:
                            nc.gpsimd.tensor_mul(PTJ[:, sb, o:o + 128],
                                                 PTJ[:, sb, o:o + 128],
                                                 tTRI[:, :])
                    return PTJ

                def finishA(h, j, PTJ):
                    nsb = 4 * (j + 1)
                    psY = ppy.tile([65, 512], F32, tag="psY")
                    for sb in range(nsb):
                        o = max(0, 128 * sb - 512 * j)
                        nc.tensor.matmul(
                            psY[:, o:512],
                            tV[:, sb, h, :],
                            PTJ[:, sb, o:512],
                            start=(sb == 0), stop=(sb == nsb - 1))
                    zrow = hp.tile([1, 512], F32, tag="zrow", bufs=2)
                    nc.vector.tensor_copy(out=zrow[:], in_=psY[64:65, :])
                    rz = hp.tile([1, 512], F32, tag="rz")
                    nc.vector.reciprocal_approx_fast(out=rz[:], in_=zrow[:])
                    cs = hp.tile([1, 512], F32R, tag="cs")
                    nc.vector.tensor_mul(cs[:], rz[:], spk[0:1, jsl[j]])
                    return psY, cs

                def finishB(h, j, psY, cs):
                    psc = pps.tile([128, 512], F32, tag="psM")
                    nc.tensor.matmul(psc[0:64, :], r(tONE[:, 0:64]), r(cs[:]),
                                     start=True, stop=True)
                    cbs = hp.tile([64, 512], F32, tag="cbs", bufs=2)
                    nc.scalar.copy(out=cbs[:], in_=psc[0:64, :])
                    g = h // 2
                    rows = slice((h % 2) * 64, (h % 2) * 64 + 64)
                    nc.vector.tensor_mul(tYL[g][rows, jsl[j]], psY[0:64, :],
                                         cbs[:])

                def outproj_mm(ms):
                    for m in ms:
                        po = pps.tile([128, 512], F32, tag="psM")
                        po2 = pps.tile([128, 512], F32, tag="psU")
                        for ne, pot in ((0, po), (1, po2)):
                            for g in range(2):
                                nc.tensor.matmul(
                                    pot[:],
                                    tYL[g][:, m * 128:(m + 1) * 128],
                                    tWO[:, g, ne * 512:(ne + 1) * 512],
                                    start=(g == 0), stop=(g == 1))
                        nc.scalar.copy(out=OO[:, m % 4, 0:512], in_=po[:])
                        nc.vector.tensor_copy(out=OO[:, m % 4, 512:1024],
                                              in_=po2[:])

                def outproj_dma(ms, eng):
                    m0, m1 = ms[0], ms[-1] + 1
                    eng.dma_start(out=out[:, m0:m1, :],
                                  in_=OO[:, m0 % 4:(m0 % 4) + (m1 - m0), :])

                order = [(0, 0), (1, 0), (2, 0), (3, 0),
                         (0, 1), (1, 1), (2, 1), (3, 1)]
                preps = {0: [(2, 0)], 1: [(3, 0)], 2: [(0, 1)], 3: [(1, 1)],
                         4: [(2, 1)], 5: [(3, 1)]}
                PTs = {}
                Ys = {}
                for gi, (h, j) in enumerate(order):
                    PTs[gi] = scores(h, j)
                    for (ph, pc) in preps.get(gi, []):
                        prep(ph, pc)
                    if gi >= 1:
                        h1, j1 = order[gi - 1]
                        Ys[gi - 1] = finishA(h1, j1, PTs.pop(gi - 1))
                    if gi >= 2:
                        h2, j2 = order[gi - 2]
                        finishB(h2, j2, *Ys.pop(gi - 2))
                        if gi == 5:
                            # query block 0 fully in tY -> compute first half
                            # of the out-projection under the j=1 scores
                            outproj_mm(range(0, 4))
                        if gi == 7:
                            # OO certainly drained by now -> near-zero wait
                            outproj_dma(range(0, 4), nc.sync)
                h1, j1 = order[7]
                Ys[7] = finishA(h1, j1, PTs.pop(7))
                h2, j2 = order[6]
                finishB(h2, j2, *Ys.pop(6))
                finishB(h1, j1, *Ys.pop(7))

                if _DEBUG:
                    nc.sync.dma_start(out=d_y0[:], in_=tY0[:])
                    nc.sync.dma_start(out=d_y1[:], in_=tY1[:])

                # ---- out projection: second half (query block 1) ----
                outproj_mm(range(4, 6))
                outproj_dma(range(4, 6), nc.gpsimd)
                outproj_mm(range(6, 8))
                outproj_dma(range(6, 8), nc.gpsimd)

    nc.finalize()
    return nc


_NC_CACHE = None


def _np_sigmoid(x):
    return 1.0 / (1.0 + np.exp(-x))


def kernel(**inputs):
    global _NC_CACHE, _S2, _S2M2, _SS, _THR, _BADJ
    x = np.asarray(inputs["x"], np.float32)
    Wqkv = np.asarray(inputs["Wqkv"], np.float32)
    bqkv = np.asarray(inputs["bqkv"], np.float32)
    Wout = np.asarray(inputs["Wout"], np.float32)
    bout = np.asarray(inputs["bout"], np.float32)
    Wimp = np.asarray(inputs["Wimp"], np.float32)
    bimp = np.asarray(inputs["bimp"], np.float32)
    Walpha = np.asarray(inputs["Walpha"], np.float32)
    balpha = np.asarray(inputs["balpha"], np.float32)
    spike_threshold = float(np.asarray(inputs["spike_threshold"]))
    log_k = np.asarray(inputs["log_k"], np.float32)
    qk_scale = float(np.asarray(inputs["qk_scale"]))

    s = _np_sigmoid(qk_scale) * 1.5
    kh = np.log1p(np.exp(log_k.astype(np.float64))) + 1e-6
    _S2 = float(s * s)
    _S2M2 = float(-2.0 * s * s)
    _SS = float(s)
    _THR = float(np.log(spike_threshold / (1.0 - spike_threshold)) - bimp[0])
    _BADJ = float(B_FIT - S_FIT)  # tanh(sf*M + (bf - sf)) = tanh(sf*(M-1)+bf)

    if _NC_CACHE is None:
        _NC_CACHE = _build()
    nc = _NC_CACHE

    # psNZ row order: 0-3 |q_h|^2, 4-7 |k_h|^2, 32-35 q_h[0]^2, 36-39 k_h[0]^2
    onesel = np.zeros((128, 4, 40), np.float32)
    for h in range(HPC):
        onesel[0:64, h, h] = 1.0
        onesel[64:128, h, 4 + h] = 1.0
        onesel[0, h, 32 + h] = 1.0
        onesel[64, h, 36 + h] = 1.0
    tri = np.triu(np.ones((128, 128), np.float32))  # keep s_loc <= t_loc
    # gf broadcast: rows 0:63 <- -gf_q(h); 64:127 <- gf_k(h)
    sel8v = np.zeros((8, 4, 128), np.float32)
    for h in range(HPC):
        sel8v[h, h, 0:64] = -1.0
        sel8v[4 + h, h, 64:128] = 1.0
    sel8v = sel8v.reshape(8, 512)

    in_maps = []
    for c in range(NCORES):
        b, hg = c // 4, c % 4
        heads = list(range(HPC * hg, HPC * hg + HPC))
        qrows = np.concatenate([np.arange(h * D, (h + 1) * D) for h in heads])
        xTb = np.ascontiguousarray(x[b].T)  # (E, T)
        wqk_rows = np.concatenate(
            [np.concatenate([Wqkv[h * D:(h + 1) * D], Wqkv[E + h * D:E + (h + 1) * D]], 0)
             for h in heads], 0)  # (512, E)
        bqk_rows = np.stack(
            [np.concatenate([bqkv[h * D:(h + 1) * D], bqkv[E + h * D:E + (h + 1) * D]], 0)
             for h in heads], 1)  # (128, 4)
        wqkT = np.ascontiguousarray(wqk_rows.T)  # (E, 512)
        wv_rows = Wqkv[2 * E:][qrows]
        bv_rows = bqkv[2 * E:][qrows]
        wvT = np.ascontiguousarray(wv_rows.T)  # (E, 256)
        wai_rows = np.concatenate([Walpha[heads], Wimp], 0)  # (5, E)
        bai = np.concatenate([balpha[heads], np.zeros(1, np.float32)], 0)
        waiT = np.ascontiguousarray(wai_rows.T)  # (E, 5)
        woT = np.ascontiguousarray(Wout[:, qrows].T).astype(np.float16)  # (256, E)
        cstv = (0.5 * bai).reshape(5, 1).astype(np.float32)
        # affine broadcast selectors vs [tA(4 rows); imp; ones]:
        #   pbu rows 0:64 = -ag = -cL*(tA+1); rows 64:128 = beta = (1-tA)/16
        #   pb4 rows = gA = cA*(tA+1)
        cav = (A_FIT / (2.0 * kh[heads])).astype(np.float32)
        clv = (A_LIN / (2.0 * kh[heads])).astype(np.float32)
        seluv = np.zeros((6, 4, 128), np.float32)
        selgv = np.zeros((6, 4, 128), np.float32)
        for i in range(HPC):
            seluv[i, i, 0:64] = -clv[i]
            seluv[5, i, 0:64] = -clv[i]
            seluv[i, i, 64:128] = -0.0625
            seluv[5, i, 64:128] = 0.0625
            selgv[i, i, :] = cav[i]
            selgv[5, i, :] = cav[i]
        seluv = seluv.reshape(6, 512)
        selgv = selgv.reshape(6, 512)
        in_maps.append({
            "xT": xTb,
            "wqk": wqkT,
            "wv": wvT,
            "wai": waiT,
            "bqk": np.ascontiguousarray(bqk_rows.astype(np.float32)),
            "bqk2": np.ascontiguousarray(
                np.concatenate([bqk_rows[0:64], bqk_rows[64:128]], 1)),
            "bvT": np.ascontiguousarray(bv_rows[None, :]),
            "wo": woT,
            "onesel": onesel,
            "cst": cstv,
            "sel8": sel8v,
            "ones1": np.ones((1, T), np.float32),
            "selu": seluv,
            "selg": selgv,
            "tri": tri.astype(__import__("ml_dtypes").bfloat16),
        })

    global _last_in_maps
    _last_in_maps = in_maps
    from concourse.bass_utils import run_bass_kernel_spmd
    res = run_bass_kernel_spmd(nc, in_maps, list(range(NCORES)))

    outv = np.zeros((B, T, E), np.float32)
    for c in range(NCORES):
        o = res.results[c]["out"].astype(np.float32)  # (128, 8, E)
        outv[c // 4] += o.transpose(1, 0, 2).reshape(T, E)
    outv += bout[None, None, :]
    return outv
